# revision 1
# baseline (speedup 1.0000x reference)
"""BEV-pool (segment-sum scatter) Trainium2 kernel for nn_BaseDepthTransform.

Design:
  Host (numpy): replicate the reference geometry -> per-point flat BEV bin id
  (depends only on the small camera matrices, not on x). Sort points by bin.
  Greedily cut the sorted stream into "groups": up to KA*128 points spanning
  < W=16 distinct bins, each group = up to KA=8 point-tiles of 128. Binary-
  decompose group tile-counts into classes {8,4,2,1} so every class has a
  uniform static schedule. Ship, per core: a bf16 feature stream, a bf16
  per-tile one-hot stream ([128 points x 16 bins], built on host), laid out
  in DMA-friendly chunks.

  Device (Bass/Tile, SPMD x8): per group, chain c matmuls
  (one-hot^T @ feats) accumulating the group's [16,80] segment sums in PSUM,
  copy PSUM->SBUF on the Scalar engine, DMA to a per-group output slot.
  Only PE + ACT + DMA are used; no dynamic addressing, no collectives.

  Host reassembly: out[group] is added into grid[base:base+16] (groups may
  share bins across classes/cores; addition commutes).
"""
import sys
sys.path.insert(0, '/opt/trn_rl_repo')

import numpy as np
import ml_dtypes

BF16 = ml_dtypes.bfloat16

# ---- static problem config (mirrors the reference) ----
IH, IW = 256, 704
FH, FW = 32, 88
D = 118
C = 80
NXg, NYg, NZg = 360, 360, 1
BXc = np.array([-53.85, -53.85, 0.0], np.float32)
DXc = np.array([0.3, 0.3, 20.0], np.float32)
NBINS = NZg * NXg * NYg  # 129600
W = 16                   # bins per group window (arbitrary base)
KA = 8                   # max tiles per group / PSUM chain
NCORES = 8
CLASSES = (8, 4, 2, 1)
# groups per DMA chunk / PSUM wave, per class (24 slots = 4 banks, 12 = 2)
CHUNK_GROUPS = {8: 24, 4: 12, 2: 12, 1: 12}  # groups per DMA chunk
PSUM_SLOTS_PER_BANK = 6  # 6 x 80 f32 = 480 of 512
WAVE = 12                # groups per PSUM wave (2 banks)

_BUILD_CACHE = {}


def _frustum():
    ds = np.arange(1.0, 60.0, 0.5, dtype=np.float32)
    xs = np.linspace(0.0, IW - 1.0, FW, dtype=np.float32)
    ys = np.linspace(0.0, IH - 1.0, FH, dtype=np.float32)
    ds_g = np.broadcast_to(ds[:, None, None], (D, FH, FW))
    xs_g = np.broadcast_to(xs[None, None, :], (D, FH, FW))
    ys_g = np.broadcast_to(ys[None, :, None], (D, FH, FW))
    return np.stack([xs_g, ys_g, ds_g], axis=-1)  # [D,FH,FW,3]


def _get_geometry(c2l_rots, c2l_trans, intrins, post_rots, post_trans,
                  extra_rots, extra_trans):
    fr = _frustum()
    pts = fr[None, None] - post_trans[:, :, None, None, None, :]
    inv_pr = np.linalg.inv(post_rots).astype(np.float32)
    pts = np.einsum('bnij,bndhwj->bndhwi', inv_pr, pts).astype(np.float32)
    pts = np.concatenate([pts[..., :2] * pts[..., 2:3], pts[..., 2:3]], axis=-1)
    combine = np.einsum(
        'bnij,bnjk->bnik', c2l_rots, np.linalg.inv(intrins).astype(np.float32)
    ).astype(np.float32)
    pts = np.einsum('bnij,bndhwj->bndhwi', combine, pts).astype(np.float32)
    pts = pts + c2l_trans[:, :, None, None, None, :]
    pts = np.einsum('bij,bndhwj->bndhwi', extra_rots, pts).astype(np.float32)
    pts = pts + extra_trans[:, None, None, None, None, :]
    return pts  # [B,N,D,FH,FW,3]


def _flat_bins(geom):
    """Per-point flat bin id (int64), -1 for dropped points."""
    coords = ((geom - (BXc - DXc / 2.0)) / DXc).astype(np.int32)
    B = coords.shape[0]
    coords = coords.reshape(B, -1, 3)
    cx, cy, cz = coords[..., 0], coords[..., 1], coords[..., 2]
    kept = (cx >= 0) & (cx < NXg) & (cy >= 0) & (cy < NYg) & (cz >= 0) & (cz < NZg)
    flat = ((cz.astype(np.int64) * NXg + cx) * NYg + cy)
    flat = np.where(kept, flat, -1)
    return flat  # [B, Np]


def _round_up(x, m):
    return ((x + m - 1) // m) * m


def _cut_groups(fk_sorted):
    """Greedy: groups of <=KA*128 points spanning < W bins, binary-decomposed
    into class segments [(cls, start, npts, base), ...] in stream order."""
    n = len(fk_sorted)
    segs = []
    i = 0
    while i < n:
        hi = np.searchsorted(fk_sorted, fk_sorted[i] + W, side='left')
        j = min(i + KA * 128, hi, n)
        npts = j - i
        base = int(fk_sorted[i])
        nt = (npts + 127) // 128
        s = i
        for c in CLASSES:
            while nt >= c:
                ln = min(c * 128, j - s)
                segs.append((c, s, ln, base))
                s += ln
                nt -= c
        i = j
    return segs


def _split_classes(segs):
    """Per class: contiguous split across cores balanced by group count,
    padded to uniform per-class counts. {cls: (percore seg lists, Gmax)}."""
    out = {}
    for c in CLASSES:
        cl = [s for s in segs if s[0] == c]
        G = len(cl)
        per = []
        for ci in range(NCORES):
            lo = (G * ci) // NCORES
            hi = (G * (ci + 1)) // NCORES
            per.append(cl[lo:hi])
        Gmax = max(1, max(len(p) for p in per))
        Gmax = _round_up(Gmax, CHUNK_GROUPS[c])
        out[c] = (per, Gmax)
    return out


def _build_core_inputs(class_split, fk_sorted, pidx_sorted, xflat_bf):
    """Build per-core input dict: per class feats + onehot streams."""
    maps = [dict() for _ in range(NCORES)]
    meta = {c: [] for c in CLASSES}  # per class: percore array of bases
    for c in CLASSES:
        per, Gmax = class_split[c]
        T = Gmax * c
        for ci in range(NCORES):
            segs = per[ci]
            feats = np.zeros((T, 128, C), BF16)
            oh = np.zeros((T, 128, W), BF16)
            bases = np.full((Gmax,), -1, np.int64)
            for gi, (_, s, ln, base) in enumerate(segs):
                bases[gi] = base
                lids = (fk_sorted[s:s + ln] - base).astype(np.int64)
                pix = pidx_sorted[s:s + ln]
                t0 = gi * c
                nt = (ln + 127) // 128
                for k in range(nt):
                    a, b = k * 128, min((k + 1) * 128, ln)
                    m = b - a
                    feats[t0 + k, :m] = xflat_bf[pix[a:b]]
                    oh[t0 + k, np.arange(m), lids[a:b]] = 1
            CH = CHUNK_GROUPS[c]
            nch = Gmax // CH
            f = feats.reshape(nch, CH * c, 128, C).transpose(0, 2, 1, 3) \
                     .reshape(nch, 128, CH * c * C)
            o8 = oh.astype(ml_dtypes.float8_e4m3)
            o = o8.reshape(nch, CH * c, 128, W).transpose(0, 2, 1, 3) \
                  .reshape(nch, 128, CH * c * W)
            maps[ci][f"feats{c}"] = np.ascontiguousarray(f)
            maps[ci][f"oh{c}"] = np.ascontiguousarray(o)
            meta[c].append(bases)
    return maps, meta


def _build_bass(shape_key):
    """shape_key: tuple of (cls, Gmax) pairs."""
    if shape_key in _BUILD_CACHE:
        return _BUILD_CACHE[shape_key]
    from concourse import bass, mybir, tile, bacc

    nc = bacc.Bacc()
    params = {}
    for c, Gmax in shape_key:
        CH = CHUNK_GROUPS[c]
        nch = Gmax // CH
        params[f"feats{c}"] = nc.declare_dram_parameter(
            f"feats{c}", [nch, 128, CH * c * C], mybir.dt.bfloat16,
            isOutput=False)
        params[f"oh{c}"] = nc.declare_dram_parameter(
            f"oh{c}", [nch, 128, CH * c * W], mybir.dt.float8e4,
            isOutput=False)
        params[f"out{c}"] = nc.declare_dram_parameter(
            f"out{c}", [W, Gmax, C], mybir.dt.float32, isOutput=True)

    def slot_off(s):
        return (s // PSUM_SLOTS_PER_BANK) * 512 + (s % PSUM_SLOTS_PER_BANK) * C

    # interleave class chunks so short small-class pipelines hide under the
    # dense class-8 stream
    chunk_order = []
    for c, Gmax in shape_key:
        nch = Gmax // CHUNK_GROUPS[c]
        for ch in range(nch):
            chunk_order.append((c, Gmax, ch, (ch + 0.5) / nch))
    chunk_order.sort(key=lambda t: t[3])

    with tile.TileContext(nc) as tc:
        with tc.tile_pool(name="fstream", bufs=5) as fpool, \
             tc.tile_pool(name="stage", bufs=8) as spool, \
             tc.tile_pool(name="psum", bufs=4, space="PSUM") as psum_pool:
            for c, Gmax, ch, _frac in chunk_order:
                CH = CHUNK_GROUPS[c]
                nch = Gmax // CH
                fchunk = fpool.tile([128, CH * c * C], mybir.dt.bfloat16,
                                    tag="fchunk")
                nc.sync.dma_start(fchunk[:], params[f"feats{c}"][ch, :, :])
                ochunk = fpool.tile([128, CH * c * W], mybir.dt.float8e4,
                                    tag="ochunk")
                nc.scalar.dma_start(ochunk[:], params[f"oh{c}"][ch, :, :])
                nwave = (CH + WAVE - 1) // WAVE
                for wv in range(nwave):
                    g0 = wv * WAVE
                    NW = min(WAVE, CH - g0)
                    nbank = NW // PSUM_SLOTS_PER_BANK
                    mega = psum_pool.tile([W, nbank * 512], mybir.dt.float32,
                                          tag="ps")
                    half = NW // 2
                    for gp in range(half):
                        ga, gb = g0 + gp, g0 + gp + half
                        oa, ob = slot_off(gp), slot_off(gp + half)
                        for k in range(c):
                            ta = ga * c + k
                            tb = gb * c + k
                            nc.tensor.matmul(
                                out=mega[:, oa:oa + C],
                                lhsT=ochunk[:, ta * W:(ta + 1) * W],
                                rhs=fchunk[:, ta * C:(ta + 1) * C],
                                start=(k == 0), stop=(k == c - 1))
                            nc.tensor.matmul(
                                out=mega[:, ob:ob + C],
                                lhsT=ochunk[:, tb * W:(tb + 1) * W],
                                rhs=fchunk[:, tb * C:(tb + 1) * C],
                                start=(k == 0), stop=(k == c - 1))
                    st = spool.tile([W, NW, C], mybir.dt.float32, tag="st")
                    src_ap = bass.AP(
                        mega[:].tensor, mega[:].offset,
                        [mega[:].ap[0], [512, nbank],
                         [C, PSUM_SLOTS_PER_BANK], [1, C]])
                    dst_ap = bass.AP(
                        st[:].tensor, st[:].offset,
                        [st[:].ap[0], [PSUM_SLOTS_PER_BANK * C, nbank],
                         [C, PSUM_SLOTS_PER_BANK], [1, C]])
                    nc.scalar.copy(dst_ap, src_ap)
                    nc.scalar.dma_start(
                        params[f"out{c}"][:, ch * CH + g0:ch * CH + g0 + NW, :],
                        st[:])
    nc.finalize()
    _BUILD_CACHE[shape_key] = nc
    return nc


def run_scheduled(x, flat, trace=False, trace_cores=None):
    """Core pipeline given precomputed flat bins; returns (grid, results)."""
    from concourse.bass_utils import run_bass_kernel_spmd

    xflat_bf = np.ascontiguousarray(x.reshape(-1, C)).astype(BF16)
    kept_idx = np.nonzero(flat >= 0)[0]
    fk = flat[kept_idx]
    order = np.argsort(fk, kind='stable')
    fk_sorted = fk[order]
    pidx_sorted = kept_idx[order]

    segs = _cut_groups(fk_sorted)
    class_split = _split_classes(segs)
    shape_key = tuple((c, class_split[c][1]) for c in CLASSES)

    maps, meta = _build_core_inputs(class_split, fk_sorted, pidx_sorted,
                                    xflat_bf)
    nc = _build_bass(shape_key)
    res = run_bass_kernel_spmd(nc, maps, core_ids=list(range(NCORES)),
                               trace=trace, trace_cores=trace_cores)

    grid = np.zeros((NBINS + W, C), np.float32)
    for c in CLASSES:
        for ci in range(NCORES):
            outs = res.results[ci][f"out{c}"]   # [W, Gmax, C]
            bases = meta[c][ci]
            for gi in range(len(bases)):
                base = bases[gi]
                if base >= 0:
                    grid[base:base + W] += outs[:, gi]
    return grid[:NBINS], res


def kernel(x, camera2lidar_rots, camera2lidar_trans, intrins, post_rots,
           post_trans, extra_rots, extra_trans):
    x = np.asarray(x, np.float32)
    B, N = x.shape[0], x.shape[1]
    assert (B, N) == (1, 6) and x.shape[2:] == (D, FH, FW, C), x.shape

    geom = _get_geometry(
        np.asarray(camera2lidar_rots, np.float32),
        np.asarray(camera2lidar_trans, np.float32),
        np.asarray(intrins, np.float32),
        np.asarray(post_rots, np.float32),
        np.asarray(post_trans, np.float32),
        np.asarray(extra_rots, np.float32),
        np.asarray(extra_trans, np.float32),
    )
    flat = _flat_bins(geom)[0]          # [Np]
    grid, _ = run_scheduled(x, flat)
    outp = grid.reshape(NXg, NYg, C).transpose(2, 0, 1)[None]  # [1,C,NX,NY]
    return np.ascontiguousarray(outp)



# revision 3
# speedup vs baseline: 1.6490x; 1.6490x over previous
"""BEV-pool (segment-sum scatter) Trainium2 kernel for nn_BaseDepthTransform.

Design (v2):
  Host (numpy): replicate the reference geometry -> per-point flat BEV bin id
  (depends only on the small camera matrices, not on x). Sort points by bin.
  Quantize the sorted feature stream to fp8-e4m3 with chained error feedback
  (carry the rounding residual into the next point, chains of L=32), which
  keeps segment-sum error at ~single-element scale. Greedily cut the sorted
  stream into "groups": up to KA*128 points spanning < W=16 distinct bins;
  binary-decompose group tile-counts into classes {8,4,2,1} and re-split
  surplus groups downward so every class count is divisible by 8 cores
  (zero cross-core padding). Ship, per core: an fp8 feature stream arranged
  as 256-point DoubleRow supertiles, and an fp8 per-supertile one-hot
  stream ([128 x 2 x 16], built on host).

  Device (Bass/Tile, SPMD x8): per group, chain c/2 DoubleRow fp8 matmuls
  (one-hot^T @ feats, 256-deep contraction per instruction) accumulating the
  group's [16,80] segment sums in PSUM; waves of <=12 groups across 2 PSUM
  banks; copy PSUM->SBUF with bf16 downcast split across the Scalar and
  Vector engines, DMA to a per-group output slot. No collectives.

  Host reassembly: out[group] is added into grid[base:base+16] (groups may
  share bins across classes/cores; addition commutes).
"""
import sys
sys.path.insert(0, '/opt/trn_rl_repo')

import numpy as np
import ml_dtypes

FP8 = ml_dtypes.float8_e4m3
BF16 = ml_dtypes.bfloat16

# ---- static problem config (mirrors the reference) ----
IH, IW = 256, 704
FH, FW = 32, 88
D = 118
C = 80
NXg, NYg, NZg = 360, 360, 1
BXc = np.array([-53.85, -53.85, 0.0], np.float32)
DXc = np.array([0.3, 0.3, 20.0], np.float32)
NBINS = NZg * NXg * NYg  # 129600
W = 16                   # bins per group window (arbitrary base)
KA = 8                   # max tiles per group
NCORES = 8
CLASSES = (8, 4, 2, 1)   # tiles per group
WAVE = 12                # groups per chunk / PSUM wave (2 banks, 6 slots each)
QCHAIN = 32              # error-feedback chain length

_BUILD_CACHE = {}


def _frustum():
    ds = np.arange(1.0, 60.0, 0.5, dtype=np.float32)
    xs = np.linspace(0.0, IW - 1.0, FW, dtype=np.float32)
    ys = np.linspace(0.0, IH - 1.0, FH, dtype=np.float32)
    ds_g = np.broadcast_to(ds[:, None, None], (D, FH, FW))
    xs_g = np.broadcast_to(xs[None, None, :], (D, FH, FW))
    ys_g = np.broadcast_to(ys[None, :, None], (D, FH, FW))
    return np.stack([xs_g, ys_g, ds_g], axis=-1)  # [D,FH,FW,3]


def _get_geometry(c2l_rots, c2l_trans, intrins, post_rots, post_trans,
                  extra_rots, extra_trans):
    fr = _frustum()
    pts = fr[None, None] - post_trans[:, :, None, None, None, :]
    inv_pr = np.linalg.inv(post_rots).astype(np.float32)
    pts = np.einsum('bnij,bndhwj->bndhwi', inv_pr, pts).astype(np.float32)
    pts = np.concatenate([pts[..., :2] * pts[..., 2:3], pts[..., 2:3]], axis=-1)
    combine = np.einsum(
        'bnij,bnjk->bnik', c2l_rots, np.linalg.inv(intrins).astype(np.float32)
    ).astype(np.float32)
    pts = np.einsum('bnij,bndhwj->bndhwi', combine, pts).astype(np.float32)
    pts = pts + c2l_trans[:, :, None, None, None, :]
    pts = np.einsum('bij,bndhwj->bndhwi', extra_rots, pts).astype(np.float32)
    pts = pts + extra_trans[:, None, None, None, None, :]
    return pts  # [B,N,D,FH,FW,3]


def _flat_bins(geom):
    """Per-point flat bin id (int64), -1 for dropped points."""
    coords = ((geom - (BXc - DXc / 2.0)) / DXc).astype(np.int32)
    B = coords.shape[0]
    coords = coords.reshape(B, -1, 3)
    cx, cy, cz = coords[..., 0], coords[..., 1], coords[..., 2]
    kept = (cx >= 0) & (cx < NXg) & (cy >= 0) & (cy < NYg) & (cz >= 0) & (cz < NZg)
    flat = ((cz.astype(np.int64) * NXg + cx) * NYg + cy)
    flat = np.where(kept, flat, -1)
    return flat  # [B, Np]


def _quantize_feedback(xs):
    """fp8-e4m3 quantize the sorted stream with per-chain error feedback.

    xs: [N, C] f32 in bin-sorted order. Returns [N, C] fp8. Rounding
    residual of each point is carried into the next point of the chain so
    segment sums see ~one element's quantization error instead of sqrt(n)."""
    N = xs.shape[0]
    L = QCHAIN
    Npad = ((N + L - 1) // L) * L
    xp = np.zeros((Npad, C), np.float32)
    xp[:N] = xs
    xp = xp.reshape(-1, L, C)
    q = np.empty_like(xp)
    carry = np.zeros((xp.shape[0], C), np.float32)
    for k in range(L):
        v = xp[:, k, :] + carry
        qk = v.astype(FP8).astype(np.float32)
        q[:, k, :] = qk
        carry = v - qk
    return q.reshape(Npad, C)[:N].astype(FP8)


def _cut_groups(fk_sorted):
    """Greedy: groups of <=KA*128 points spanning < W bins, binary-decomposed
    into class segments [(cls, start, npts, base), ...] in stream order."""
    n = len(fk_sorted)
    segs = []
    i = 0
    while i < n:
        hi = np.searchsorted(fk_sorted, fk_sorted[i] + W, side='left')
        j = min(i + KA * 128, hi, n)
        npts = j - i
        base = int(fk_sorted[i])
        nt = (npts + 127) // 128
        s = i
        for c in CLASSES:
            while nt >= c:
                ln = min(c * 128, j - s)
                segs.append((c, s, ln, base))
                s += ln
                nt -= c
        i = j
    return segs


def _balance_classes(segs):
    """Split surplus groups into the next class down so every class count is
    divisible by NCORES; pad class 1 with dummy groups. Returns
    {cls: percore lists} with uniform per-core counts and no padding groups
    except <=7 class-1 dummies globally."""
    cls_lists = {c: [s for s in segs if s[0] == c] for c in CLASSES}
    for c in (8, 4, 2):
        lst = cls_lists[c]
        r = len(lst) % NCORES
        if r:
            moved = lst[-r:]
            del lst[-r:]
            h = (c // 2) * 128
            for (_, s, ln, base) in moved:
                cls_lists[c // 2].append((c // 2, s, h, base))
                cls_lists[c // 2].append((c // 2, s + h, ln - h, base))
    r = len(cls_lists[1]) % NCORES
    if r:
        cls_lists[1] += [(1, 0, 0, NBINS)] * (NCORES - r)
    out = {}
    for c in CLASSES:
        lst = cls_lists[c]
        G = len(lst)
        assert G % NCORES == 0
        per = [lst[(G * ci) // NCORES:(G * (ci + 1)) // NCORES]
               for ci in range(NCORES)]
        out[c] = (per, max(1, G // NCORES))
    return out


def _build_core_inputs(class_split, fk_sorted, q8_sorted):
    """Per-core input dict: per class fp8 feats + onehot streams in
    DoubleRow supertile layout [128 | st, plane, :]."""
    maps = [dict() for _ in range(NCORES)]
    meta = {}
    for c in CLASSES:
        per, Gc = class_split[c]
        T = Gc * c
        bases_all = []
        for ci in range(NCORES):
            segs = per[ci]
            feats = np.zeros((T, 128, C), FP8)
            oh = np.zeros((T, 128, W), FP8)
            bases = np.full((Gc,), NBINS, np.int64)
            for gi, (_, s, ln, base) in enumerate(segs):
                if ln == 0:
                    continue
                bases[gi] = base
                lids = (fk_sorted[s:s + ln] - base).astype(np.int64)
                t0 = gi * c
                nt = (ln + 127) // 128
                for k in range(nt):
                    a, b = k * 128, min((k + 1) * 128, ln)
                    m = b - a
                    feats[t0 + k, :m] = q8_sorted[s + a:s + b]
                    oh[t0 + k, np.arange(m), lids[a:b]] = 1
            if c >= 2:
                ST = T // 2
                f = feats.reshape(ST, 2, 128, C).transpose(2, 0, 1, 3) \
                         .reshape(128, ST * 2 * C)
                o = oh.reshape(ST, 2, 128, W).transpose(2, 0, 1, 3) \
                      .reshape(128, ST * 2 * W)
            else:
                f = feats.transpose(1, 0, 2).reshape(128, T * C)
                o = oh.transpose(1, 0, 2).reshape(128, T * W)
            maps[ci][f"feats{c}"] = np.ascontiguousarray(f)
            maps[ci][f"oh{c}"] = np.ascontiguousarray(o)
            bases_all.append(bases)
        meta[c] = bases_all
    return maps, meta


def _build_bass(shape_key):
    """shape_key: tuple of (cls, Gc) pairs."""
    if shape_key in _BUILD_CACHE:
        return _BUILD_CACHE[shape_key]
    from concourse import bass, mybir, tile, bacc

    nc = bacc.Bacc()
    params = {}
    for c, Gc in shape_key:
        stg = max(1, c // 2)
        nplane = 2 if c >= 2 else 1
        params[f"feats{c}"] = nc.declare_dram_parameter(
            f"feats{c}", [128, Gc * stg * nplane * C], mybir.dt.float8e4,
            isOutput=False)
        params[f"oh{c}"] = nc.declare_dram_parameter(
            f"oh{c}", [128, Gc * stg * nplane * W], mybir.dt.float8e4,
            isOutput=False)
        params[f"out{c}"] = nc.declare_dram_parameter(
            f"out{c}", [W, Gc, C], mybir.dt.bfloat16, isOutput=True)

    # chunks of <=WAVE groups; interleave classes so short small-class
    # pipelines hide under the dense class-8 stream
    chunk_order = []
    for c, Gc in shape_key:
        nch = (Gc + WAVE - 1) // WAVE
        for ch in range(nch):
            g0 = ch * WAVE
            nw = min(WAVE, Gc - g0)
            chunk_order.append((c, g0, nw, (ch + 0.5) / nch))
    chunk_order.sort(key=lambda t: t[3])

    DR = mybir.MatmulPerfMode.DoubleRow

    with tile.TileContext(nc) as tc:
        with tc.tile_pool(name="fstream", bufs=6) as fpool, \
             tc.tile_pool(name="stage", bufs=8) as spool, \
             tc.tile_pool(name="psum", bufs=4, space="PSUM") as psum_pool:
            for c, g0, NW, _frac in chunk_order:
                stg = max(1, c // 2)
                if c >= 2:
                    fchunk = fpool.tile([128, WAVE * stg, 2, C],
                                        mybir.dt.float8e4, tag=f"f{c}")
                    nc.sync.dma_start(
                        fchunk[:, :NW * stg, :, :],
                        params[f"feats{c}"][:, g0 * stg * 2 * C:
                                            (g0 + NW) * stg * 2 * C])
                    ochunk = fpool.tile([128, WAVE * stg, 2, W],
                                        mybir.dt.float8e4, tag=f"o{c}")
                    nc.scalar.dma_start(
                        ochunk[:, :NW * stg, :, :],
                        params[f"oh{c}"][:, g0 * stg * 2 * W:
                                         (g0 + NW) * stg * 2 * W])
                else:
                    fchunk = fpool.tile([128, WAVE, C], mybir.dt.float8e4,
                                        tag="f1")
                    nc.sync.dma_start(
                        fchunk[:, :NW, :],
                        params[f"feats{c}"][:, g0 * C:(g0 + NW) * C])
                    ochunk = fpool.tile([128, WAVE, W], mybir.dt.float8e4,
                                        tag="o1")
                    nc.scalar.dma_start(
                        ochunk[:, :NW, :],
                        params[f"oh{c}"][:, g0 * W:(g0 + NW) * W])

                # PSUM wave: bank0 holds chains 0..n0-1, bank1 n0..NW-1
                mega = psum_pool.tile([W, 1024], mybir.dt.float32, tag="ps")
                half = NW // 2
                n0 = NW - half  # bank0 count (>= bank1 count)

                def chain(gl, slot_off):
                    if c >= 2:
                        for k in range(stg):
                            st = gl * stg + k
                            nc.tensor.matmul(
                                out=mega[:, slot_off:slot_off + C],
                                lhsT=ochunk[:, st, :, :],
                                rhs=fchunk[:, st, :, :],
                                start=(k == 0), stop=(k == stg - 1),
                                perf_mode=DR)
                    else:
                        nc.tensor.matmul(
                            out=mega[:, slot_off:slot_off + C],
                            lhsT=ochunk[:, gl, :],
                            rhs=fchunk[:, gl, :],
                            start=True, stop=True)

                # interleave pairs of chains across the two banks
                for gp in range(half):
                    chain(gp, gp * C)
                    chain(n0 + gp, 512 + gp * C)
                if n0 > half:
                    chain(half, half * C)

                st = spool.tile([W, WAVE, C], mybir.dt.bfloat16, tag="st")
                nc.scalar.copy(st[:, :n0, :], mega[:, :n0 * C])
                if NW > n0:
                    nc.vector.tensor_copy(st[:, n0:NW, :],
                                          mega[:, 512:512 + (NW - n0) * C])
                nc.scalar.dma_start(
                    params[f"out{c}"][:, g0:g0 + NW, :], st[:, :NW, :])
    nc.finalize()
    _BUILD_CACHE[shape_key] = nc
    return nc


def run_scheduled(x, flat, trace=False, trace_cores=None):
    """Core pipeline given precomputed flat bins; returns (grid, results)."""
    from concourse.bass_utils import run_bass_kernel_spmd

    xflat = np.ascontiguousarray(x.reshape(-1, C)).astype(np.float32)
    kept_idx = np.nonzero(flat >= 0)[0]
    fk = flat[kept_idx]
    order = np.argsort(fk, kind='stable')
    fk_sorted = fk[order]
    q8_sorted = _quantize_feedback(xflat[kept_idx[order]])

    segs = _cut_groups(fk_sorted)
    class_split = _balance_classes(segs)
    shape_key = tuple((c, class_split[c][1]) for c in CLASSES)

    maps, meta = _build_core_inputs(class_split, fk_sorted, q8_sorted)
    nc = _build_bass(shape_key)
    res = run_bass_kernel_spmd(nc, maps, core_ids=list(range(NCORES)),
                               trace=trace, trace_cores=trace_cores)

    grid = np.zeros((NBINS + W, C), np.float32)
    for c in CLASSES:
        Gc = class_split[c][1]
        idx = np.arange(W)[None, :]  # [1, W]
        for ci in range(NCORES):
            outs = np.asarray(res.results[ci][f"out{c}"],
                              dtype=np.float32)     # [W, Gc, C]
            bases = meta[c][ci]                      # [Gc]
            rows = (bases[:, None] + idx).ravel()    # [Gc*W]
            np.add.at(grid, rows, outs.transpose(1, 0, 2).reshape(-1, C))
    return grid[:NBINS], res


def kernel(x, camera2lidar_rots, camera2lidar_trans, intrins, post_rots,
           post_trans, extra_rots, extra_trans):
    x = np.asarray(x, np.float32)
    B, N = x.shape[0], x.shape[1]
    assert (B, N) == (1, 6) and x.shape[2:] == (D, FH, FW, C), x.shape

    geom = _get_geometry(
        np.asarray(camera2lidar_rots, np.float32),
        np.asarray(camera2lidar_trans, np.float32),
        np.asarray(intrins, np.float32),
        np.asarray(post_rots, np.float32),
        np.asarray(post_trans, np.float32),
        np.asarray(extra_rots, np.float32),
        np.asarray(extra_trans, np.float32),
    )
    flat = _flat_bins(geom)[0]          # [Np]
    grid, _ = run_scheduled(x, flat)
    outp = grid.reshape(NXg, NYg, C).transpose(2, 0, 1)[None]  # [1,C,NX,NY]
    return np.ascontiguousarray(outp)


# revision 5
# speedup vs baseline: 1.9649x; 1.1916x over previous
"""BEV-pool (segment-sum scatter) Trainium2 kernel for nn_BaseDepthTransform.

Design (v2):
  Host (numpy): replicate the reference geometry -> per-point flat BEV bin id
  (depends only on the small camera matrices, not on x). Sort points by bin.
  Quantize the sorted feature stream to fp8-e4m3 with chained error feedback
  (carry the rounding residual into the next point, chains of L=32), which
  keeps segment-sum error at ~single-element scale. Greedily cut the sorted
  stream into "groups": up to KA*128 points spanning < W=16 distinct bins;
  binary-decompose group tile-counts into classes {8,4,2,1} and re-split
  surplus groups downward so every class count is divisible by 8 cores
  (zero cross-core padding). Ship, per core: an fp8 feature stream arranged
  as 256-point DoubleRow supertiles, and an fp8 per-supertile one-hot
  stream ([128 x 2 x 16], built on host).

  Device (Bass/Tile, SPMD x8): per group, chain c/2 DoubleRow fp8 matmuls
  (one-hot^T @ feats, 256-deep contraction per instruction) accumulating the
  group's [16,80] segment sums in PSUM; waves of <=12 groups across 2 PSUM
  banks; copy PSUM->SBUF with bf16 downcast split across the Scalar and
  Vector engines, DMA to a per-group output slot. No collectives.

  Host reassembly: out[group] is added into grid[base:base+16] (groups may
  share bins across classes/cores; addition commutes).
"""
import sys
sys.path.insert(0, '/opt/trn_rl_repo')

import numpy as np
import ml_dtypes

FP8 = ml_dtypes.float8_e4m3
BF16 = ml_dtypes.bfloat16

# ---- static problem config (mirrors the reference) ----
IH, IW = 256, 704
FH, FW = 32, 88
D = 118
C = 80
NXg, NYg, NZg = 360, 360, 1
BXc = np.array([-53.85, -53.85, 0.0], np.float32)
DXc = np.array([0.3, 0.3, 20.0], np.float32)
NBINS = NZg * NXg * NYg  # 129600
W = 16                   # bins per group window (arbitrary base)
KA = 8                   # max tiles per group
NCORES = 8
CLASSES = (8, 4, 2, 1)   # tiles per group
WAVE = 12                # groups per chunk / PSUM wave (2 banks, 6 slots each)
QCHAIN = 32              # error-feedback chain length

_BUILD_CACHE = {}


def _frustum():
    ds = np.arange(1.0, 60.0, 0.5, dtype=np.float32)
    xs = np.linspace(0.0, IW - 1.0, FW, dtype=np.float32)
    ys = np.linspace(0.0, IH - 1.0, FH, dtype=np.float32)
    ds_g = np.broadcast_to(ds[:, None, None], (D, FH, FW))
    xs_g = np.broadcast_to(xs[None, None, :], (D, FH, FW))
    ys_g = np.broadcast_to(ys[None, :, None], (D, FH, FW))
    return np.stack([xs_g, ys_g, ds_g], axis=-1)  # [D,FH,FW,3]


def _get_geometry(c2l_rots, c2l_trans, intrins, post_rots, post_trans,
                  extra_rots, extra_trans):
    fr = _frustum()
    pts = fr[None, None] - post_trans[:, :, None, None, None, :]
    inv_pr = np.linalg.inv(post_rots).astype(np.float32)
    pts = np.einsum('bnij,bndhwj->bndhwi', inv_pr, pts).astype(np.float32)
    pts = np.concatenate([pts[..., :2] * pts[..., 2:3], pts[..., 2:3]], axis=-1)
    combine = np.einsum(
        'bnij,bnjk->bnik', c2l_rots, np.linalg.inv(intrins).astype(np.float32)
    ).astype(np.float32)
    pts = np.einsum('bnij,bndhwj->bndhwi', combine, pts).astype(np.float32)
    pts = pts + c2l_trans[:, :, None, None, None, :]
    pts = np.einsum('bij,bndhwj->bndhwi', extra_rots, pts).astype(np.float32)
    pts = pts + extra_trans[:, None, None, None, None, :]
    return pts  # [B,N,D,FH,FW,3]


def _flat_bins(geom):
    """Per-point flat bin id (int64), -1 for dropped points."""
    coords = ((geom - (BXc - DXc / 2.0)) / DXc).astype(np.int32)
    B = coords.shape[0]
    coords = coords.reshape(B, -1, 3)
    cx, cy, cz = coords[..., 0], coords[..., 1], coords[..., 2]
    kept = (cx >= 0) & (cx < NXg) & (cy >= 0) & (cy < NYg) & (cz >= 0) & (cz < NZg)
    flat = ((cz.astype(np.int64) * NXg + cx) * NYg + cy)
    flat = np.where(kept, flat, -1)
    return flat  # [B, Np]


def _quantize_feedback(xs):
    """fp8-e4m3 quantize the sorted stream with per-chain error feedback.

    xs: [N, C] f32 in bin-sorted order. Returns [N, C] fp8. Rounding
    residual of each point is carried into the next point of the chain so
    segment sums see ~one element's quantization error instead of sqrt(n)."""
    N = xs.shape[0]
    L = QCHAIN
    Npad = ((N + L - 1) // L) * L
    xp = np.zeros((Npad, C), np.float32)
    xp[:N] = xs
    xp = xp.reshape(-1, L, C)
    q = np.empty_like(xp)
    carry = np.zeros((xp.shape[0], C), np.float32)
    for k in range(L):
        v = xp[:, k, :] + carry
        qk = v.astype(FP8).astype(np.float32)
        q[:, k, :] = qk
        carry = v - qk
    return q.reshape(Npad, C)[:N].astype(FP8)


def _cut_groups(fk_sorted):
    """Greedy: groups of <=KA*128 points spanning < W bins, binary-decomposed
    into class segments [(cls, start, npts, base), ...] in stream order."""
    n = len(fk_sorted)
    segs = []
    i = 0
    while i < n:
        hi = np.searchsorted(fk_sorted, fk_sorted[i] + W, side='left')
        j = min(i + KA * 128, hi, n)
        npts = j - i
        base = int(fk_sorted[i])
        nt = (npts + 127) // 128
        s = i
        for c in CLASSES:
            while nt >= c:
                ln = min(c * 128, j - s)
                segs.append((c, s, ln, base))
                s += ln
                nt -= c
        i = j
    return segs


def _balance_classes(segs):
    """Split surplus groups into the next class down so every class count is
    divisible by NCORES; pad class 1 with dummy groups. Returns
    {cls: percore lists} with uniform per-core counts and no padding groups
    except <=7 class-1 dummies globally."""
    cls_lists = {c: [s for s in segs if s[0] == c] for c in CLASSES}
    for c in (8, 4, 2):
        lst = cls_lists[c]
        r = len(lst) % NCORES
        if r:
            moved = lst[-r:]
            del lst[-r:]
            h = (c // 2) * 128
            for (_, s, ln, base) in moved:
                cls_lists[c // 2].append((c // 2, s, h, base))
                cls_lists[c // 2].append((c // 2, s + h, ln - h, base))
    r = len(cls_lists[1]) % NCORES
    if r:
        cls_lists[1] += [(1, 0, 0, NBINS)] * (NCORES - r)
    out = {}
    for c in CLASSES:
        lst = cls_lists[c]
        G = len(lst)
        assert G % NCORES == 0
        per = [lst[(G * ci) // NCORES:(G * (ci + 1)) // NCORES]
               for ci in range(NCORES)]
        out[c] = (per, max(1, G // NCORES))
    return out


def _build_core_inputs(class_split, fk_sorted, q8_sorted):
    """Per-core input dict: per class fp8 feats + onehot streams in
    DoubleRow supertile layout [128 | st, plane, :]."""
    maps = [dict() for _ in range(NCORES)]
    meta = {}
    for c in CLASSES:
        per, Gc = class_split[c]
        T = Gc * c
        bases_all = []
        for ci in range(NCORES):
            segs = per[ci]
            feats = np.zeros((T, 128, C), FP8)
            oh = np.zeros((T, 128, W), FP8)
            bases = np.full((Gc,), NBINS, np.int64)
            for gi, (_, s, ln, base) in enumerate(segs):
                if ln == 0:
                    continue
                bases[gi] = base
                lids = (fk_sorted[s:s + ln] - base).astype(np.int64)
                t0 = gi * c
                nt = (ln + 127) // 128
                for k in range(nt):
                    a, b = k * 128, min((k + 1) * 128, ln)
                    m = b - a
                    feats[t0 + k, :m] = q8_sorted[s + a:s + b]
                    oh[t0 + k, np.arange(m), lids[a:b]] = 1
            if c >= 2:
                ST = T // 2
                f = feats.reshape(ST, 2, 128, C).transpose(2, 0, 1, 3) \
                         .reshape(128, ST * 2 * C)
                o = oh.reshape(ST, 2, 128, W).transpose(2, 0, 1, 3) \
                      .reshape(128, ST * 2 * W)
            else:
                f = feats.transpose(1, 0, 2).reshape(128, T * C)
                o = oh.transpose(1, 0, 2).reshape(128, T * W)
            maps[ci][f"feats{c}"] = np.ascontiguousarray(f)
            maps[ci][f"oh{c}"] = np.ascontiguousarray(o)
            bases_all.append(bases)
        meta[c] = bases_all
    return maps, meta


def _build_bass(shape_key):
    """shape_key: tuple of (cls, Gc) pairs."""
    if shape_key in _BUILD_CACHE:
        return _BUILD_CACHE[shape_key]
    from concourse import bass, mybir, tile, bacc

    nc = bacc.Bacc()
    params = {}
    for c, Gc in shape_key:
        stg = max(1, c // 2)
        nplane = 2 if c >= 2 else 1
        params[f"feats{c}"] = nc.declare_dram_parameter(
            f"feats{c}", [128, Gc * stg * nplane * C], mybir.dt.float8e4,
            isOutput=False)
        params[f"oh{c}"] = nc.declare_dram_parameter(
            f"oh{c}", [128, Gc * stg * nplane * W], mybir.dt.float8e4,
            isOutput=False)
        params[f"out{c}"] = nc.declare_dram_parameter(
            f"out{c}", [W, Gc, C], mybir.dt.bfloat16, isOutput=True)

    # chunks of <=CHUNK groups (2 PSUM waves); interleave classes so short
    # small-class pipelines hide under the dense class-8 stream
    CHUNK = 2 * WAVE
    chunk_order = []
    for c, Gc in shape_key:
        nch = (Gc + CHUNK - 1) // CHUNK
        for ch in range(nch):
            g0 = ch * CHUNK
            nw = min(CHUNK, Gc - g0)
            chunk_order.append((c, g0, nw, (ch + 0.5) / nch))
    chunk_order.sort(key=lambda t: t[3])

    DR = mybir.MatmulPerfMode.DoubleRow

    with tile.TileContext(nc) as tc:
        with tc.tile_pool(name="fstream", bufs=3) as fpool, \
             tc.tile_pool(name="stage", bufs=6) as spool, \
             tc.tile_pool(name="psum", bufs=4, space="PSUM") as psum_pool:
            for ci_, (c, g0, NC, _frac) in enumerate(chunk_order):
                stg = max(1, c // 2)
                # alternate DMA issue between the two hwdge queues
                eng_f = nc.sync if ci_ % 2 == 0 else nc.scalar
                eng_o = nc.scalar if ci_ % 2 == 0 else nc.sync
                if c >= 2:
                    fchunk = fpool.tile([128, CHUNK * stg, 2, C],
                                        mybir.dt.float8e4, tag=f"f{c}")
                    eng_f.dma_start(
                        fchunk[:, :NC * stg, :, :],
                        params[f"feats{c}"][:, g0 * stg * 2 * C:
                                            (g0 + NC) * stg * 2 * C])
                    ochunk = fpool.tile([128, CHUNK * stg, 2, W],
                                        mybir.dt.float8e4, tag=f"o{c}")
                    eng_o.dma_start(
                        ochunk[:, :NC * stg, :, :],
                        params[f"oh{c}"][:, g0 * stg * 2 * W:
                                         (g0 + NC) * stg * 2 * W])
                else:
                    fchunk = fpool.tile([128, CHUNK, C], mybir.dt.float8e4,
                                        tag="f1")
                    eng_f.dma_start(
                        fchunk[:, :NC, :],
                        params[f"feats{c}"][:, g0 * C:(g0 + NC) * C])
                    ochunk = fpool.tile([128, CHUNK, W], mybir.dt.float8e4,
                                        tag="o1")
                    eng_o.dma_start(
                        ochunk[:, :NC, :],
                        params[f"oh{c}"][:, g0 * W:(g0 + NC) * W])

                stage = spool.tile([W, CHUNK, C], mybir.dt.bfloat16, tag="st")
                nwave = (NC + WAVE - 1) // WAVE
                for wv in range(nwave):
                    w0 = wv * WAVE          # chunk-local first group
                    NW = min(WAVE, NC - w0)
                    # PSUM wave: bank0 holds chains 0..n0-1, bank1 the rest
                    mega = psum_pool.tile([W, 1024], mybir.dt.float32,
                                          tag="ps")
                    half = NW // 2
                    n0 = NW - half  # bank0 count (>= bank1 count)

                    def chain(gl, slot_off):
                        if c >= 2:
                            for k in range(stg):
                                sti = (w0 + gl) * stg + k
                                nc.tensor.matmul(
                                    out=mega[:, slot_off:slot_off + C],
                                    lhsT=ochunk[:, sti, :, :],
                                    rhs=fchunk[:, sti, :, :],
                                    start=(k == 0), stop=(k == stg - 1),
                                    perf_mode=DR)
                        else:
                            nc.tensor.matmul(
                                out=mega[:, slot_off:slot_off + C],
                                lhsT=ochunk[:, w0 + gl, :],
                                rhs=fchunk[:, w0 + gl, :],
                                start=True, stop=True)

                    # interleave pairs of chains across the two banks
                    for gp in range(half):
                        chain(gp, gp * C)
                        chain(n0 + gp, 512 + gp * C)
                    if n0 > half:
                        chain(half, half * C)

                    # PSUM -> SBUF copy w/ bf16 downcast on DVE (Pool can't
                    # read PSUM; keep the hwdge engines free for DMA issue)
                    eng_c = nc.vector
                    if n0 == 6 and NW == 12:
                        src_ap = bass.AP(
                            mega[:].tensor, mega[:].offset,
                            [mega[:].ap[0], [512, 2], [C, 6], [1, C]])
                        s_sl = stage[:, w0:w0 + NW, :]
                        dst_ap = bass.AP(
                            s_sl.tensor, s_sl.offset,
                            [s_sl.ap[0], [6 * C, 2], [C, 6], [1, C]])
                        eng_c.tensor_copy(dst_ap, src_ap)
                    else:
                        eng_c.tensor_copy(stage[:, w0:w0 + n0, :],
                                          mega[:, :n0 * C])
                        if NW > n0:
                            eng_c.tensor_copy(
                                stage[:, w0 + n0:w0 + NW, :],
                                mega[:, 512:512 + (NW - n0) * C])
                eng_o.dma_start(
                    params[f"out{c}"][:, g0:g0 + NC, :], stage[:, :NC, :])
    nc.finalize()
    _BUILD_CACHE[shape_key] = nc
    return nc


def run_scheduled(x, flat, trace=False, trace_cores=None):
    """Core pipeline given precomputed flat bins; returns (grid, results)."""
    from concourse.bass_utils import run_bass_kernel_spmd

    xflat = np.ascontiguousarray(x.reshape(-1, C)).astype(np.float32)
    kept_idx = np.nonzero(flat >= 0)[0]
    fk = flat[kept_idx]
    order = np.argsort(fk, kind='stable')
    fk_sorted = fk[order]
    q8_sorted = _quantize_feedback(xflat[kept_idx[order]])

    segs = _cut_groups(fk_sorted)
    class_split = _balance_classes(segs)
    shape_key = tuple((c, class_split[c][1]) for c in CLASSES)

    maps, meta = _build_core_inputs(class_split, fk_sorted, q8_sorted)
    nc = _build_bass(shape_key)
    res = run_bass_kernel_spmd(nc, maps, core_ids=list(range(NCORES)),
                               trace=trace, trace_cores=trace_cores)

    grid = np.zeros((NBINS + W, C), np.float32)
    for c in CLASSES:
        Gc = class_split[c][1]
        idx = np.arange(W)[None, :]  # [1, W]
        for ci in range(NCORES):
            outs = np.asarray(res.results[ci][f"out{c}"],
                              dtype=np.float32)     # [W, Gc, C]
            bases = meta[c][ci]                      # [Gc]
            rows = (bases[:, None] + idx).ravel()    # [Gc*W]
            np.add.at(grid, rows, outs.transpose(1, 0, 2).reshape(-1, C))
    return grid[:NBINS], res


def kernel(x, camera2lidar_rots, camera2lidar_trans, intrins, post_rots,
           post_trans, extra_rots, extra_trans):
    x = np.asarray(x, np.float32)
    B, N = x.shape[0], x.shape[1]
    assert (B, N) == (1, 6) and x.shape[2:] == (D, FH, FW, C), x.shape

    geom = _get_geometry(
        np.asarray(camera2lidar_rots, np.float32),
        np.asarray(camera2lidar_trans, np.float32),
        np.asarray(intrins, np.float32),
        np.asarray(post_rots, np.float32),
        np.asarray(post_trans, np.float32),
        np.asarray(extra_rots, np.float32),
        np.asarray(extra_trans, np.float32),
    )
    flat = _flat_bins(geom)[0]          # [Np]
    grid, _ = run_scheduled(x, flat)
    outp = grid.reshape(NXg, NYg, C).transpose(2, 0, 1)[None]  # [1,C,NX,NY]
    return np.ascontiguousarray(outp)
